# revision 6
# baseline (speedup 1.0000x reference)
"""GroupAttention sparse-attention kernel for 8 trn2 NeuronCores (v2).

Math (derived + numerically verified against the reference):
  - The tridiagonal mask means each softmax row has >=1 finite entries at
    j=i+-1, or is fully uniform 1/S ("caseB" rows u_i=1).
  - neibor = v0 + (vBB-v0)*u u^T off-band (rank-1), band overwritten with
    d_sup (super/sub) and d_main (diag).
  - g[i,j] = exp(cum[j]-cum[i]) + 1e-9 for j>i (symmetric), diag d_main,
    where cum = exclusive prefix-sum of ell = log(d_sup+1e-9).
  - scores use A~ = wk^T wq:  s_next[i] = xn_i . (A~^T xn_{i+1}) / 512.
SPMD: core 2b -> batch b rows [0,1024); core 2b+1 -> batch b reversed
(problem is reversal-covariant), host un-reverses. bq/bk/beta zeros and
gamma ones per the spec, so they are folded away. Outputs are bf16 on
device (tolerance is 2e-2), cast to f32 on host.
"""

import numpy as np
from contextlib import ExitStack

B, S, D = 4, 2048, 1024
NT = 8          # 128-row blocks per core (half of S/128)
HALF = S // 2

_cache = {}


def _build():
    import concourse.bass as bass
    import concourse.bacc as bacc
    import concourse.mybir as mybir
    from concourse.tile import TileContext

    f32 = mybir.dt.float32
    bf16 = mybir.dt.bfloat16
    i32 = mybir.dt.int32
    AF = mybir.ActivationFunctionType
    OP = mybir.AluOpType

    nc = bacc.Bacc("TRN2", target_bir_lowering=False)

    # ---------------- I/O ----------------
    x_in = nc.dram_tensor("x", [S, D], bf16, kind="ExternalInput")
    eospad = nc.dram_tensor("eospad", [S + 2], i32, kind="ExternalInput")
    prior_t = nc.dram_tensor("prior", [1], f32, kind="ExternalInput")
    wq_in = nc.dram_tensor("wq", [D, D], bf16, kind="ExternalInput")
    wk_in = nc.dram_tensor("wk", [D, D], bf16, kind="ExternalInput")
    lt_in = nc.dram_tensor("lt128", [128, 128], f32, kind="ExternalInput")
    wup_in = nc.dram_tensor("wupi", [128, 128], i32, kind="ExternalInput")
    eyei_in = nc.dram_tensor("eyei", [128, 128], i32, kind="ExternalInput")
    eyef_in = nc.dram_tensor("eyef", [128, 128], f32, kind="ExternalInput")
    bmf_in = nc.dram_tensor("bmf", [128, 3, 131], f32, kind="ExternalInput")
    bmi_in = nc.dram_tensor("bmi", [128, 131], i32, kind="ExternalInput")
    ones_in = nc.dram_tensor("onesb", [128, 1], bf16, kind="ExternalInput")
    usclv_in = nc.dram_tensor("usclv", [S], f32, kind="ExternalInput")
    ucol_in = nc.dram_tensor("ucol", [HALF], f32, kind="ExternalInput")
    out_nb = nc.dram_tensor("out_nb", [HALF, S], bf16, kind="ExternalOutput")
    out_g = nc.dram_tensor("out_g", [HALF, S], bf16, kind="ExternalOutput")

    C_SQ9 = float(np.sqrt(np.float32(1e-9)))                    # sqrt(1e-9)
    C_SBB = float(np.sqrt(np.float32((1.0 / S) ** 2 + 1e-9)))   # caseB diag sqrt

    def bcast(dram_ap, n):
        return bass.AP(tensor=dram_ap.tensor, offset=dram_ap.offset,
                       ap=[[0, 128], [1, n]])

    with TileContext(nc) as tc, ExitStack() as ctx:
        # ---------------- pools (whole-kernel lifetime) ----------------
        consts = ctx.enter_context(tc.tile_pool(name="consts", bufs=1))
        vec = ctx.enter_context(tc.tile_pool(name="vec", bufs=28))
        col = ctx.enter_context(tc.tile_pool(name="col", bufs=12))
        bigrow = ctx.enter_context(tc.tile_pool(name="bigrow", bufs=1))
        at_pool = ctx.enter_context(tc.tile_pool(name="atp", bufs=1))
        xnt_pool = ctx.enter_context(tc.tile_pool(name="xntp", bufs=1))
        dram = ctx.enter_context(tc.tile_pool(name="dram", bufs=1, space="DRAM"))

        # ---------------- consts into SBUF ----------------
        lt128 = consts.tile([128, 128], f32)
        nc.sync.dma_start(out=lt128, in_=lt_in[:, :])
        wup_i = consts.tile([128, 128], i32)
        nc.sync.dma_start(out=wup_i, in_=wup_in[:, :])
        eye_i = consts.tile([128, 128], i32)
        nc.sync.dma_start(out=eye_i, in_=eyei_in[:, :])
        eye_f = consts.tile([128, 128], f32)
        nc.sync.dma_start(out=eye_f, in_=eyef_in[:, :])
        bmf = consts.tile([128, 3, 131], f32)
        nc.sync.dma_start(out=bmf, in_=bmf_in[:, :, :])
        bmi = consts.tile([128, 131], i32)
        nc.sync.dma_start(out=bmi, in_=bmi_in[:, :])
        ones_b = consts.tile([128, 1], bf16)
        nc.sync.dma_start(out=ones_b, in_=ones_in[:, :])
        pr_col = consts.tile([128, 1], f32)
        nc.sync.dma_start(out=pr_col, in_=bcast(prior_t[:], 1))
        omp_col = consts.tile([128, 1], f32)  # 1 - prior
        nc.vector.tensor_scalar(omp_col, pr_col, -1.0, 1.0, OP.mult, OP.add)
        v0_col = consts.tile([128, 1], f32)
        nc.vector.tensor_scalar(v0_col, omp_col, C_SQ9, None, OP.mult)
        nc.vector.tensor_tensor(v0_col, v0_col, pr_col, OP.add)
        neg9 = consts.tile([128, 16], f32)
        nc.vector.memset(neg9, -1.0e9)
        # register const bias columns used by activation(bias=float)
        for ci, cval in enumerate((0.0, 1e-9, 1e-5)):
            cc = consts.tile([128, 1], f32, name=f"cc{ci}", tag=f"cc{ci}")
            nc.vector.memset(cc, cval)
            nc.const_aps.aps[(f32, cval)] = cc[:, :]

        urow = bigrow.tile([128, S], f32, name="urow", tag="urow")
        nc.sync.dma_start(out=urow, in_=bcast(usclv_in[:], S))
        ucol_t = col.tile([128, 8], f32, name="ucolt", tag="ucolt")
        nc.sync.dma_start(
            out=ucol_t, in_=ucol_in[0:HALF].rearrange("(t p) -> p t", p=128)
        )

        # ---------------- DRAM scratch ----------------
        xn_d = dram.tile([S, D], bf16)
        sn_d = dram.tile([S], f32)
        sp_d = dram.tile([S], f32)
        cum_d = dram.tile([S], f32)
        dsup_d = dram.tile([S + 1], f32)        # [0]=0, [1+i]=d_sup[i]
        dmain_d = dram.tile([S], f32)

        # ============ phase 1: weights; A~^T = wk^T wq (bf16); LN ============
        with ExitStack() as p1:
            wpool = p1.enter_context(tc.tile_pool(name="wpool", bufs=1))
            xpool = p1.enter_context(tc.tile_pool(name="xpool", bufs=4))
            xbpool = p1.enter_context(tc.tile_pool(name="xbpool", bufs=4))
            stpool = p1.enter_context(tc.tile_pool(name="stpool", bufs=6))
            psA = p1.enter_context(tc.tile_pool(name="psA", bufs=2, space="PSUM"))

            wqb = wpool.tile([128, 8, D], bf16)
            wkb = wpool.tile([128, 8, D], bf16)
            for dt in range(8):
                nc.sync.dma_start(out=wkb[:, dt, :],
                                  in_=wk_in[dt * 128:(dt + 1) * 128, :])
                nc.sync.dma_start(out=wqb[:, dt, :],
                                  in_=wq_in[dt * 128:(dt + 1) * 128, :])
            xts = []
            for it in range(16):
                xt = xpool.tile([128, D], bf16)
                nc.sync.dma_start(out=xt, in_=x_in[it * 128:(it + 1) * 128, :])
                xts.append(xt)

            at_sb = at_pool.tile([128, 8, D], bf16)  # AT[p,ft,e] = A~[f,e]
            for ft in range(8):
                ps = psA.tile([128, D], f32)
                for dt in range(8):
                    for c in range(2):
                        nc.tensor.matmul(
                            ps[:, c * 512:(c + 1) * 512],
                            wkb[:, dt, ft * 128:(ft + 1) * 128],
                            wqb[:, dt, c * 512:(c + 1) * 512],
                            start=(dt == 0),
                            stop=(dt == 7),
                        )
                nc.scalar.copy(out=at_sb[:, ft, :], in_=ps[:, :])

            # --- LN per 128-row tile, write bf16 normalized x to DRAM ---
            for it in range(16):
                xt = xts[it]
                stats = stpool.tile([128, 2, 6], f32)
                nc.vector.bn_stats(out=stats[:, 0, :], in_=xt[:, 0:512])
                nc.vector.bn_stats(out=stats[:, 1, :], in_=xt[:, 512:1024])
                mv = stpool.tile([128, 2], f32)
                nc.vector.bn_aggr(out=mv, in_=stats)
                sdv = stpool.tile([128, 1], f32)
                nc.scalar.activation(sdv, mv[:, 1:2], AF.Sqrt, bias=1e-5)
                rstd = stpool.tile([128, 1], f32)
                nc.vector.reciprocal(rstd, sdv)
                xbt = xbpool.tile([128, D], bf16)
                nc.vector.tensor_scalar(
                    xbt, xt, mv[:, 0:1], rstd, OP.subtract, OP.mult
                )
                nc.sync.dma_start(out=xn_d[it * 128:(it + 1) * 128, :], in_=xbt)

        # ============ phase 2: transpose; z; band dot-products ============
        xnt = xnt_pool.tile([128, 8, S], bf16)   # xnt[p,ft,i] = xn[i, ft*128+p]
        for ft in range(8):
            nc.sync.dma_start(
                out=xnt[:, ft, :], in_=xn_d[:, ft * 128:(ft + 1) * 128],
                transpose=True,
            )

        # nb rank-1 tiles (only need eos/prior) — DMA-out slack during z MMs
        with ExitStack() as pnb:
            nbpool = pnb.enter_context(tc.tile_pool(name="nbpool", bufs=3))
            for t in range(NT):
                nb = nbpool.tile([128, S], bf16)
                nc.vector.tensor_scalar(
                    nb, urow, ucol_t[:, t:t + 1], v0_col, OP.mult, OP.add
                )
                nc.sync.dma_start(out=out_nb[t * 128:(t + 1) * 128, :], in_=nb)

        with ExitStack() as p2:
            zpool = p2.enter_context(tc.tile_pool(name="zpool", bufs=2))
            p1pool = p2.enter_context(tc.tile_pool(name="p1pool", bufs=2))
            p2pool = p2.enter_context(tc.tile_pool(name="p2pool", bufs=8))
            rows = p2.enter_context(tc.tile_pool(name="rows", bufs=2))
            psZ = p2.enter_context(tc.tile_pool(name="psZ", bufs=2, space="PSUM"))
            psN = p2.enter_context(tc.tile_pool(name="psN", bufs=1, space="PSUM"))

            ps_n = psN.tile([1, S], f32, tag="psrow", name="ps_n")
            p2tiles = []
            for et in range(8):
                zb = zpool.tile([128, S], bf16)
                for half in range(2):
                    ps = psZ.tile([128, 1024], f32)
                    for ft in range(8):
                        for c in range(2):
                            off = half * 1024 + c * 512
                            nc.tensor.matmul(
                                ps[:, c * 512:(c + 1) * 512],
                                at_sb[:, ft, et * 128:(et + 1) * 128],
                                xnt[:, ft, off:off + 512],
                                start=(ft == 0),
                                stop=(ft == 7),
                            )
                    nc.scalar.copy(out=zb[:, half * 1024:(half + 1) * 1024],
                                   in_=ps)
                pt1 = p1pool.tile([128, S], bf16)
                nc.vector.tensor_tensor(
                    pt1[:, 0:S - 1], xnt[:, et, 0:S - 1], zb[:, 1:S], OP.mult
                )
                pt2 = p2pool.tile([128, S], bf16)
                nc.vector.tensor_tensor(
                    pt2[:, 1:S], xnt[:, et, 1:S], zb[:, 0:S - 1], OP.mult
                )
                p2tiles.append(pt2)
                for c in range(4):
                    nc.tensor.matmul(
                        ps_n[0:1, c * 512:(c + 1) * 512],
                        ones_b,
                        pt1[:, c * 512:(c + 1) * 512],
                        start=(et == 0),
                        stop=(et == 7),
                    )
            row_n = rows.tile([1, S], f32)
            nc.scalar.mul(row_n, ps_n[0:1, :], 1.0 / 512.0)
            nc.sync.dma_start(out=sn_d[:], in_=row_n)

            ps_p = psN.tile([1, S], f32, tag="psrow", name="ps_p")
            for et in range(8):
                for c in range(4):
                    nc.tensor.matmul(
                        ps_p[0:1, c * 512:(c + 1) * 512],
                        ones_b,
                        p2tiles[et][:, c * 512:(c + 1) * 512],
                        start=(et == 0),
                        stop=(et == 7),
                    )
            row_p = rows.tile([1, S], f32)
            nc.scalar.mul(row_p, ps_p[0:1, :], 1.0 / 512.0)
            nc.sync.dma_start(out=sp_d[:], in_=row_p)

        # ============ phase 3: band math in [128,16] layout ============
        def v16():
            return vec.tile([128, 16], f32, tag="v16", name="v16")

        def rd16(dtensor, off):  # dram vec [off:off+2048] -> [128,16] row-major
            return dtensor[off:off + S].rearrange("(p c) -> p c", c=16)

        sn = v16()
        nc.sync.dma_start(out=sn, in_=rd16(sn_d, 0))
        sp = v16()
        nc.sync.dma_start(out=sp, in_=rd16(sp_d, 0))
        hn_i = vec.tile([128, 16], i32)
        nc.sync.dma_start(out=hn_i, in_=rd16(eospad[:], 2))
        hp_i = vec.tile([128, 16], i32)
        nc.sync.dma_start(out=hp_i, in_=rd16(eospad[:], 0))
        hn = v16()
        nc.vector.tensor_copy(out=hn, in_=hn_i)
        hp = v16()
        nc.vector.tensor_copy(out=hp, in_=hp_i)

        sne = v16()
        nc.vector.select(sne, hn_i, sn, neg9)
        spe = v16()
        nc.vector.select(spe, hp_i, sp, neg9)
        m = v16()
        nc.vector.tensor_tensor(m, sne, spe, OP.max)
        en = v16()
        nc.vector.tensor_tensor(en, sne, m, OP.subtract)
        nc.scalar.activation(en, en, AF.Exp)
        ep = v16()
        nc.vector.tensor_tensor(ep, spe, m, OP.subtract)
        nc.scalar.activation(ep, ep, AF.Exp)
        zs = v16()
        nc.vector.tensor_tensor(zs, en, ep, OP.add)
        rz = v16()
        nc.vector.reciprocal(rz, zs)
        nn = v16()
        nc.vector.tensor_tensor(nn, en, rz, OP.mult)
        npv = v16()
        nc.vector.tensor_tensor(npv, ep, rz, OP.mult)
        # caseB flag u = (1-hn)*(1-hp); blend N with uniform 1/S
        t1 = v16()
        nc.vector.tensor_scalar(t1, hn, -1.0, 1.0, OP.mult, OP.add)
        t2 = v16()
        nc.vector.tensor_scalar(t2, hp, -1.0, 1.0, OP.mult, OP.add)
        cb = v16()
        nc.vector.tensor_tensor(cb, t1, t2, OP.mult)
        omcb = v16()
        nc.vector.tensor_scalar(omcb, cb, -1.0, 1.0, OP.mult, OP.add)
        cbS = v16()
        nc.vector.tensor_scalar(cbS, cb, 1.0 / S, None, OP.mult)
        for nv in (nn, npv):
            nc.vector.tensor_tensor(nv, nv, omcb, OP.mult)
            nc.vector.tensor_tensor(nv, nv, cbS, OP.add)
        # Np shifted by +1 (value at i+1)
        npsh = v16()
        nc.vector.memset(npsh, 0.0)
        nc.vector.tensor_copy(out=npsh[:, 0:15], in_=npv[:, 1:16])
        nc.sync.dma_start(out=npsh[0:127, 15:16], in_=npv[1:128, 0:1])
        msup = v16()
        nc.vector.tensor_tensor(msup, nn, npsh, OP.mult)
        # d_sup = prior + (1-prior)*exp(0.5*ln(msup+1e-9))
        dsup = v16()
        nc.scalar.activation(dsup, msup, AF.Ln, bias=1e-9)
        nc.scalar.activation(dsup, dsup, AF.Exp, scale=0.5)
        nc.vector.tensor_scalar(dsup, dsup, omp_col, pr_col, OP.mult, OP.add)
        # d_main = prior + (1-prior)*(c1 + (c2-c1)*cb)
        dmain = v16()
        nc.vector.tensor_scalar(dmain, cb, C_SBB - C_SQ9, C_SQ9, OP.mult, OP.add)
        nc.vector.tensor_scalar(dmain, dmain, omp_col, pr_col, OP.mult, OP.add)
        # ell, prefix sums
        ell = v16()
        nc.scalar.activation(ell, dsup, AF.Ln, bias=1e-9)
        zv16 = v16()
        nc.vector.memset(zv16, 0.0)
        incl = v16()
        nc.vector.tensor_tensor_scan(incl, ell, zv16, 0.0, OP.add, OP.add)
        excl = v16()
        nc.vector.tensor_tensor(excl, incl, ell, OP.subtract)
        with ExitStack() as p3:
            ps3 = p3.enter_context(tc.tile_pool(name="ps3", bufs=1, space="PSUM"))
            ps_c = ps3.tile([128, 1], f32)
            nc.tensor.matmul(ps_c, lt128, incl[:, 15:16], start=True, stop=True)
            cp_col = col.tile([128, 1], f32)
            nc.vector.tensor_copy(out=cp_col, in_=ps_c)
        cum = v16()
        nc.vector.tensor_scalar(cum, excl, cp_col, None, OP.add)

        def wr16(dtensor, off, src):
            nc.sync.dma_start(
                out=dtensor[off:off + S].rearrange("(p c) -> p c", c=16), in_=src
            )

        wr16(cum_d, 0, cum)
        wr16(dsup_d, 1, dsup)
        wr16(dmain_d, 0, dmain)

        # ============ phase 4: g tiles + band-window strips ============
        with ExitStack() as p4:
            outp = p4.enter_context(tc.tile_pool(name="outp", bufs=3))
            gwin = p4.enter_context(tc.tile_pool(name="gwin", bufs=4))
            strp = p4.enter_context(tc.tile_pool(name="strp", bufs=4))

            cumrow = bigrow.tile([128, S], f32, name="cumrow", tag="cumrow")
            nc.sync.dma_start(out=cumrow, in_=bcast(cum_d[:], S))
            cumcol = col.tile([128, 8], f32)
            nc.sync.dma_start(
                out=cumcol, in_=cum_d[0:HALF].rearrange("(t p) -> p t", p=128)
            )
            negcum = col.tile([128, 8], f32)
            nc.vector.tensor_scalar(negcum, cumcol, -1.0, None, OP.mult)
            dmain_c = col.tile([128, 8], f32)
            nc.sync.dma_start(
                out=dmain_c, in_=dmain_d[0:HALF].rearrange("(t p) -> p t", p=128)
            )
            dsup_c = col.tile([128, 8], f32)
            nc.sync.dma_start(
                out=dsup_c, in_=dsup_d[1:1 + HALF].rearrange("(t p) -> p t", p=128)
            )
            dsupsh_c = col.tile([128, 8], f32)
            nc.sync.dma_start(
                out=dsupsh_c, in_=dsup_d[0:HALF].rearrange("(t p) -> p t", p=128)
            )
            # dsup_d[0] is an uninitialized pad slot; its only consumer is
            # (p=0,t=0) which the sub-diag mask zeroes — but NaN*0=NaN, so
            # scrub it.
            nc.vector.memset(dsupsh_c[0:1, 0:1], 0.0)

            for t in range(NT):
                r0 = t * 128
                gb = outp.tile([128, S], bf16)
                if t > 0:
                    nc.scalar.activation(gb[:, 0:r0], cumrow[:, 0:r0], AF.Exp,
                                         scale=-1.0, bias=cumcol[:, t:t + 1])
                nc.scalar.activation(gb[:, r0 + 128:S], cumrow[:, r0 + 128:S],
                                     AF.Exp, scale=1.0, bias=negcum[:, t:t + 1])
                # window: e2 (lower valid) in place, e1 (upper valid) via mask
                nc.scalar.activation(gb[:, r0:r0 + 128], cumrow[:, r0:r0 + 128],
                                     AF.Exp, scale=-1.0, bias=cumcol[:, t:t + 1])
                e1 = gwin.tile([128, 128], bf16)
                nc.scalar.activation(e1, cumrow[:, r0:r0 + 128], AF.Exp,
                                     scale=1.0, bias=negcum[:, t:t + 1])
                nc.vector.copy_predicated(gb[:, r0:r0 + 128], wup_i, e1)
                dmb = gwin.tile([128, 128], bf16)
                nc.vector.tensor_scalar(dmb, eye_f, dmain_c[:, t:t + 1], None,
                                        OP.mult)
                nc.vector.copy_predicated(gb[:, r0:r0 + 128], eye_i, dmb)
                nc.vector.tensor_scalar(gb, gb, 1.0e-9, None, OP.add)
                nc.sync.dma_start(out=out_g[r0:r0 + 128, :], in_=gb)

                # nb band strip [128,130] at cols [c0, c0+130)
                c0 = r0 - 1 if t > 0 else 0
                mo = 0 if t > 0 else 1
                nbw = strp.tile([128, 130], bf16, name="nbw", tag="nbw")
                nc.vector.tensor_scalar(
                    nbw, urow[:, c0:c0 + 130], ucol_t[:, t:t + 1], v0_col,
                    OP.mult, OP.add
                )
                bv = strp.tile([128, 130], bf16, name="bv", tag="bv")
                nc.vector.tensor_scalar(bv, bmf[:, 0, mo:mo + 130],
                                        dmain_c[:, t:t + 1], None, OP.mult)
                b2 = strp.tile([128, 130], bf16, name="b2", tag="b2")
                nc.vector.tensor_scalar(b2, bmf[:, 1, mo:mo + 130],
                                        dsup_c[:, t:t + 1], None, OP.mult)
                nc.vector.tensor_tensor(bv, bv, b2, OP.add)
                nc.vector.tensor_scalar(b2, bmf[:, 2, mo:mo + 130],
                                        dsupsh_c[:, t:t + 1], None, OP.mult)
                nc.vector.tensor_tensor(bv, bv, b2, OP.add)
                nc.vector.copy_predicated(nbw, bmi[:, mo:mo + 130], bv)
                nc.sync.dma_start(out=out_nb[r0:r0 + 128, c0:c0 + 130], in_=nbw)

    nc.compile()
    return nc


def _consts():
    import ml_dtypes
    k = np.arange(128)
    lt = (k[:, None] < k[None, :]).astype(np.float32)       # lt[k,p]=k<p
    wup_i = (k[None, :] > k[:, None]).astype(np.int32)      # wup[p,w]=w>p
    eye_i = (k[None, :] == k[:, None]).astype(np.int32)
    eye_f = eye_i.astype(np.float32)
    q = np.arange(131)
    eye131 = (q[None, :] == k[:, None] + 1).astype(np.float32)
    sup131 = (q[None, :] == k[:, None] + 2).astype(np.float32)
    sub131 = (q[None, :] == k[:, None]).astype(np.float32)
    bmf = np.stack([eye131, sup131, sub131], axis=1)        # [128,3,131]
    bmi = (eye131 + sup131 + sub131).astype(np.int32)
    ones = np.ones((128, 1), dtype=ml_dtypes.bfloat16)
    return lt, wup_i, eye_i, eye_f, bmf, bmi, ones


def kernel(context, eos_mask, prior, wq, bq, wk, bk, gamma, beta):
    import ml_dtypes
    from concourse.bass_utils import run_bass_kernel_spmd

    if "nc" not in _cache:
        _cache["nc"] = _build()
    nc = _cache["nc"]

    bf = ml_dtypes.bfloat16
    context = np.asarray(context, np.float32)
    eos_mask = np.asarray(eos_mask, np.int32)
    prior = np.asarray(prior, np.float32)
    wqb = np.asarray(wq, np.float32).astype(bf)
    wkb = np.asarray(wk, np.float32).astype(bf)
    lt, wup_i, eye_i, eye_f, bmf, bmi, ones = _consts()

    pr = np.float32(prior[0])
    v0 = pr + (1 - pr) * np.float32(np.sqrt(np.float32(1e-9)))
    vbb = pr + (1 - pr) * np.float32(np.sqrt(np.float32((1.0 / S) ** 2 + 1e-9)))
    dv = np.float32(vbb - v0)

    in_maps = []
    for c in range(8):
        b, h = c // 2, c % 2
        x = context[b] if h == 0 else context[b][::-1]
        eo = eos_mask[b] if h == 0 else eos_mask[b][::-1]
        eop = np.zeros(S + 2, np.int32)
        eop[1:S + 1] = eo
        u = ((1 - eop[2:S + 2]) * (1 - eop[0:S])).astype(np.float32)
        in_maps.append({
            "x": np.ascontiguousarray(x).astype(bf),
            "eospad": eop,
            "prior": prior,
            "wq": wqb, "wk": wkb,
            "lt128": lt, "wupi": wup_i, "eyei": eye_i, "eyef": eye_f,
            "bmf": bmf, "bmi": bmi, "onesb": ones,
            "usclv": dv * u,
            "ucol": u[0:HALF],
        })

    bkr = run_bass_kernel_spmd(nc, in_maps, core_ids=list(range(8)))
    _cache["last_bkr"] = bkr

    g_out = np.empty((B, S, S), np.float32)
    nb_out = np.empty((B, S, S), np.float32)
    for c in range(8):
        b, h = c // 2, c % 2
        rg = np.asarray(bkr.results[c]["out_g"]).astype(np.float32)
        rn = np.asarray(bkr.results[c]["out_nb"]).astype(np.float32)
        if h == 0:
            g_out[b, :HALF] = rg
            nb_out[b, :HALF] = rn
        else:
            g_out[b, HALF:] = rg[::-1, ::-1]
            nb_out[b, HALF:] = rn[::-1, ::-1]
    return g_out, nb_out


# revision 10
# speedup vs baseline: 1.0736x; 1.0736x over previous
"""GroupAttention sparse-attention kernel for 8 trn2 NeuronCores (v3).

Math (derived + numerically verified against the reference):
  - The tridiagonal mask means each softmax row has >=1 finite entries at
    j=i+-1, or is fully uniform 1/S ("caseB" rows u_i=1).
  - neibor = v0 + (vBB-v0)*u u^T off-band (rank-1), band overwritten with
    d_sup (super/sub) and d_main (diag).
  - g[i,j] = exp(cum[j]-cum[i]) + 1e-9 for j>i (symmetric), diag d_main,
    where cum = exclusive prefix-sum of ell = log(d_sup+1e-9).
  - scores use A~ = wk^T wq and LayerNorm folded into the epilogue:
      xn_i A xn_j = rstd_i rstd_j (xr_i A xr_j - mu_j (xr_i.w1)
                                   - mu_i (w2.xr_j) + mu_i mu_j s11)
    with w1 = A 1, w2 = 1^T A, s11 = 1^T A 1 (A = A~^T), so the device
    only ever touches RAW x — transposes start at t=0 with no LN chain.
    mu/rstd/w1/w2/s11 are computed exactly on the host.
SPMD: core 2b -> batch b rows [0,1024); core 2b+1 -> batch b reversed
(problem is reversal-covariant), host un-reverses. bq/bk/beta zeros and
gamma ones per the spec, so they are folded away. Outputs are bf16 on
device (tolerance is 2e-2), cast to f32 on host.
"""

import numpy as np
from contextlib import ExitStack

B, S, D = 4, 2048, 1024
NT = 8          # 128-row blocks per core (half of S/128)
HALF = S // 2

_cache = {}


def _build():
    import concourse.bass as bass
    import concourse.bacc as bacc
    import concourse.mybir as mybir
    from concourse.tile import TileContext

    f32 = mybir.dt.float32
    bf16 = mybir.dt.bfloat16
    i32 = mybir.dt.int32
    AF = mybir.ActivationFunctionType
    OP = mybir.AluOpType

    nc = bacc.Bacc("TRN2", target_bir_lowering=False)

    # ---------------- I/O ----------------
    x_in = nc.dram_tensor("x", [S, D], bf16, kind="ExternalInput")
    eospad = nc.dram_tensor("eospad", [S + 2], i32, kind="ExternalInput")
    prior_t = nc.dram_tensor("prior", [1], f32, kind="ExternalInput")
    s11_t = nc.dram_tensor("s11", [1], f32, kind="ExternalInput")
    wq_in = nc.dram_tensor("wq", [D, D], bf16, kind="ExternalInput")
    wk_in = nc.dram_tensor("wk", [D, D], bf16, kind="ExternalInput")
    w1_in = nc.dram_tensor("w1", [D], bf16, kind="ExternalInput")
    w2_in = nc.dram_tensor("w2", [D], bf16, kind="ExternalInput")
    mu_in = nc.dram_tensor("mupad", [S + 2], f32, kind="ExternalInput")
    rs_in = nc.dram_tensor("rstdpad", [S + 2], f32, kind="ExternalInput")
    lt_in = nc.dram_tensor("lt128", [128, 128], f32, kind="ExternalInput")
    wup_in = nc.dram_tensor("wupi", [128, 128], i32, kind="ExternalInput")
    eyei_in = nc.dram_tensor("eyei", [128, 128], i32, kind="ExternalInput")
    eyef_in = nc.dram_tensor("eyef", [128, 128], f32, kind="ExternalInput")
    bmf_in = nc.dram_tensor("bmf", [128, 3, 131], f32, kind="ExternalInput")
    bmi_in = nc.dram_tensor("bmi", [128, 131], i32, kind="ExternalInput")
    ones_in = nc.dram_tensor("onesb", [128, 1], bf16, kind="ExternalInput")
    usclv_in = nc.dram_tensor("usclv", [S], f32, kind="ExternalInput")
    ucol_in = nc.dram_tensor("ucol", [HALF], f32, kind="ExternalInput")
    out_nb = nc.dram_tensor("out_nb", [HALF, S], bf16, kind="ExternalOutput")
    out_g = nc.dram_tensor("out_g", [HALF, S], bf16, kind="ExternalOutput")

    C_SQ9 = float(np.sqrt(np.float32(1e-9)))                    # sqrt(1e-9)
    C_SBB = float(np.sqrt(np.float32((1.0 / S) ** 2 + 1e-9)))   # caseB diag sqrt

    def bcast(dram_ap, n):
        return bass.AP(tensor=dram_ap.tensor, offset=dram_ap.offset,
                       ap=[[0, 128], [1, n]])

    with TileContext(nc) as tc, ExitStack() as ctx:
        # ---------------- pools (whole-kernel lifetime) ----------------
        consts = ctx.enter_context(tc.tile_pool(name="consts", bufs=1))
        vec = ctx.enter_context(tc.tile_pool(name="vec", bufs=44))
        col = ctx.enter_context(tc.tile_pool(name="col", bufs=12))
        bigrow = ctx.enter_context(tc.tile_pool(name="bigrow", bufs=1))
        at_pool = ctx.enter_context(tc.tile_pool(name="atp", bufs=1))
        xrt_pool = ctx.enter_context(tc.tile_pool(name="xrtp", bufs=1))
        dram = ctx.enter_context(tc.tile_pool(name="dram", bufs=1, space="DRAM"))

        # ---------------- consts into SBUF ----------------
        lt128 = consts.tile([128, 128], f32)
        nc.sync.dma_start(out=lt128, in_=lt_in[:, :])
        wup_i = consts.tile([128, 128], i32)
        nc.sync.dma_start(out=wup_i, in_=wup_in[:, :])
        eye_i = consts.tile([128, 128], i32)
        nc.sync.dma_start(out=eye_i, in_=eyei_in[:, :])
        eye_f = consts.tile([128, 128], f32)
        nc.sync.dma_start(out=eye_f, in_=eyef_in[:, :])
        bmf = consts.tile([128, 3, 131], f32)
        nc.sync.dma_start(out=bmf, in_=bmf_in[:, :, :])
        bmi = consts.tile([128, 131], i32)
        nc.sync.dma_start(out=bmi, in_=bmi_in[:, :])
        ones_b = consts.tile([128, 1], bf16)
        nc.sync.dma_start(out=ones_b, in_=ones_in[:, :])
        pr_col = consts.tile([128, 1], f32)
        nc.sync.dma_start(out=pr_col, in_=bcast(prior_t[:], 1))
        s11_col = consts.tile([128, 1], f32)
        nc.sync.dma_start(out=s11_col, in_=bcast(s11_t[:], 1))
        w1_sb = consts.tile([128, 8], bf16)
        nc.sync.dma_start(
            out=w1_sb, in_=w1_in[0:D].rearrange("(t p) -> p t", p=128))
        w2_sb = consts.tile([128, 8], bf16)
        nc.sync.dma_start(
            out=w2_sb, in_=w2_in[0:D].rearrange("(t p) -> p t", p=128))
        omp_col = consts.tile([128, 1], f32)  # 1 - prior
        nc.vector.tensor_scalar(omp_col, pr_col, -1.0, 1.0, OP.mult, OP.add)
        v0_col = consts.tile([128, 1], f32)
        nc.vector.tensor_scalar(v0_col, omp_col, C_SQ9, None, OP.mult)
        nc.vector.tensor_tensor(v0_col, v0_col, pr_col, OP.add)
        neg9 = consts.tile([128, 16], f32)
        nc.vector.memset(neg9, -1.0e9)
        # register const bias columns used by activation(bias=float)
        for ci, cval in enumerate((0.0, 1e-9)):
            cc = consts.tile([128, 1], f32, name=f"cc{ci}", tag=f"cc{ci}")
            nc.vector.memset(cc, cval)
            nc.const_aps.aps[(f32, cval)] = cc[:, :]
        zrow = consts.tile([1, 2], f32)
        nc.vector.memset(zrow, 0.0)

        urow = bigrow.tile([128, S], f32, name="urow", tag="urow")
        nc.sync.dma_start(out=urow, in_=bcast(usclv_in[:], S))
        ucol_t = col.tile([128, 8], f32, name="ucolt", tag="ucolt")
        nc.sync.dma_start(
            out=ucol_t, in_=ucol_in[0:HALF].rearrange("(t p) -> p t", p=128)
        )

        # ---------------- DRAM scratch ----------------
        a1_d = dram.tile([S], f32)              # xr_i A xr_{i+1}
        a2_d = dram.tile([S], f32)              # xr_i A xr_{i-1}
        br_d = dram.tile([S], f32)              # xr_i . w1
        cr_d = dram.tile([S + 2], f32)          # [1+i] = w2 . xr_i
        cum_d = dram.tile([S], f32)
        dsup_d = dram.tile([S + 1], f32)        # [0]=pad, [1+i]=d_sup[i]
        dmain_d = dram.tile([S], f32)
        # zero cr_d's pad slots (read via shifted rd16 loads; disjoint from
        # the crow row write, so these can issue early)
        nc.sync.dma_start(out=cr_d[0:1], in_=zrow[0:1, 0:1])
        nc.sync.dma_start(out=cr_d[S + 1:S + 2], in_=zrow[0:1, 1:2])

        # ============ phase 1: weights; raw-x transposes; A~ ============
        # transpose raw x straight from the input — no LN dependency
        xrT = xrt_pool.tile([128, 8, S], bf16)   # xrT[p,ft,i] = x[i, ft*128+p]
        with ExitStack() as p1:
            wpool = p1.enter_context(tc.tile_pool(name="wpool", bufs=1))
            psA = p1.enter_context(tc.tile_pool(name="psA", bufs=2, space="PSUM"))

            wqb = wpool.tile([128, 8, D], bf16)
            wkb = wpool.tile([128, 8, D], bf16)
            for dt in range(8):
                nc.sync.dma_start(out=wkb[:, dt, :],
                                  in_=wk_in[dt * 128:(dt + 1) * 128, :])
                nc.sync.dma_start(out=wqb[:, dt, :],
                                  in_=wq_in[dt * 128:(dt + 1) * 128, :])
            for ft in range(8):
                nc.sync.dma_start(
                    out=xrT[:, ft, :], in_=x_in[:, ft * 128:(ft + 1) * 128],
                    transpose=True,
                )

            at_sb = at_pool.tile([128, 8, D], bf16)  # AT[p,ft,e] = A~[f,e]
            for ft in range(8):
                ps = psA.tile([128, D], f32)
                for dt in range(8):
                    for c in range(2):
                        nc.tensor.matmul(
                            ps[:, c * 512:(c + 1) * 512],
                            wkb[:, dt, ft * 128:(ft + 1) * 128],
                            wqb[:, dt, c * 512:(c + 1) * 512],
                            start=(dt == 0),
                            stop=(dt == 7),
                        )
                nc.scalar.copy(out=at_sb[:, ft, :], in_=ps[:, :])

        # ============ phase 2: brow/crow; z; band dot-products ============
        # nb rank-1 tiles (only need eos/prior) — DMA-out slack during z MMs
        with ExitStack() as pnb:
            nbpool = pnb.enter_context(tc.tile_pool(name="nbpool", bufs=3))
            for t in range(NT):
                nb = nbpool.tile([128, S], bf16)
                nc.vector.tensor_scalar(
                    nb, urow, ucol_t[:, t:t + 1], v0_col, OP.mult, OP.add
                )
                nc.sync.dma_start(out=out_nb[t * 128:(t + 1) * 128, :], in_=nb)

        with ExitStack() as p2:
            zpool = p2.enter_context(tc.tile_pool(name="zpool", bufs=2))
            p1pool = p2.enter_context(tc.tile_pool(name="p1pool", bufs=2))
            p2pool = p2.enter_context(tc.tile_pool(name="p2pool", bufs=8))
            rows = p2.enter_context(tc.tile_pool(name="rows", bufs=2))
            psZ = p2.enter_context(tc.tile_pool(name="psZ", bufs=2, space="PSUM"))
            psN = p2.enter_context(tc.tile_pool(name="psN", bufs=1, space="PSUM"))

            # brow = xr.w1 and crow = w2.xr rows (need only xrT + host w1/w2)
            for nm, wcol, dst, doff in (("br", w1_sb, br_d, 0),
                                        ("cr", w2_sb, cr_d, 1)):
                ps_r = psN.tile([1, S], f32, tag="psrow", name=f"ps_{nm}")
                for eb in range(8):
                    for c in range(4):
                        nc.tensor.matmul(
                            ps_r[0:1, c * 512:(c + 1) * 512],
                            wcol[:, eb:eb + 1],
                            xrT[:, eb, c * 512:(c + 1) * 512],
                            start=(eb == 0),
                            stop=(eb == 7),
                        )
                row_r = rows.tile([1, S], f32, tag="rowr", name=f"row_{nm}")
                nc.scalar.copy(out=row_r, in_=ps_r[0:1, :])
                nc.sync.dma_start(out=dst[doff:doff + S], in_=row_r)

            ps_n = psN.tile([1, S], f32, tag="psrow", name="ps_n")
            p2tiles = []
            for et in range(8):
                zb = zpool.tile([128, S], bf16)
                for half in range(2):
                    ps = psZ.tile([128, 1024], f32)
                    for ft in range(8):
                        for c in range(2):
                            off = half * 1024 + c * 512
                            nc.tensor.matmul(
                                ps[:, c * 512:(c + 1) * 512],
                                at_sb[:, ft, et * 128:(et + 1) * 128],
                                xrT[:, ft, off:off + 512],
                                start=(ft == 0),
                                stop=(ft == 7),
                            )
                    nc.scalar.copy(out=zb[:, half * 1024:(half + 1) * 1024],
                                   in_=ps)
                pt1 = p1pool.tile([128, S], bf16)
                nc.vector.tensor_tensor(
                    pt1[:, 0:S - 1], xrT[:, et, 0:S - 1], zb[:, 1:S], OP.mult
                )
                pt2 = p2pool.tile([128, S], bf16)
                nc.vector.tensor_tensor(
                    pt2[:, 1:S], xrT[:, et, 1:S], zb[:, 0:S - 1], OP.mult
                )
                p2tiles.append(pt2)
                for c in range(4):
                    nc.tensor.matmul(
                        ps_n[0:1, c * 512:(c + 1) * 512],
                        ones_b,
                        pt1[:, c * 512:(c + 1) * 512],
                        start=(et == 0),
                        stop=(et == 7),
                    )
            row_n = rows.tile([1, S], f32, tag="rowr", name="row_n")
            nc.scalar.copy(out=row_n, in_=ps_n[0:1, :])
            nc.sync.dma_start(out=a1_d[:], in_=row_n)

            ps_p = psN.tile([1, S], f32, tag="psrow", name="ps_p")
            for et in range(8):
                for c in range(4):
                    nc.tensor.matmul(
                        ps_p[0:1, c * 512:(c + 1) * 512],
                        ones_b,
                        p2tiles[et][:, c * 512:(c + 1) * 512],
                        start=(et == 0),
                        stop=(et == 7),
                    )
            row_p = rows.tile([1, S], f32, tag="rowr", name="row_p")
            nc.scalar.copy(out=row_p, in_=ps_p[0:1, :])
            nc.sync.dma_start(out=a2_d[:], in_=row_p)

        # ============ phase 3: band math in [128,16] layout ============
        def v16():
            return vec.tile([128, 16], f32, tag="v16", name="v16")

        def rd16(dtensor, off):  # dram vec [off:off+2048] -> [128,16] row-major
            return dtensor[off:off + S].rearrange("(p c) -> p c", c=16)

        a1 = v16()
        nc.sync.dma_start(out=a1, in_=rd16(a1_d, 0))
        a2 = v16()
        nc.sync.dma_start(out=a2, in_=rd16(a2_d, 0))
        br = v16()
        nc.sync.dma_start(out=br, in_=rd16(br_d, 0))
        cp1 = v16()
        nc.sync.dma_start(out=cp1, in_=rd16(cr_d, 2))
        cm1 = v16()
        nc.sync.dma_start(out=cm1, in_=rd16(cr_d, 0))
        mu = v16()
        nc.sync.dma_start(out=mu, in_=rd16(mu_in[:], 1))
        mup = v16()
        nc.sync.dma_start(out=mup, in_=rd16(mu_in[:], 2))
        mum = v16()
        nc.sync.dma_start(out=mum, in_=rd16(mu_in[:], 0))
        rs = v16()
        nc.sync.dma_start(out=rs, in_=rd16(rs_in[:], 1))
        rsp = v16()
        nc.sync.dma_start(out=rsp, in_=rd16(rs_in[:], 2))
        rsm = v16()
        nc.sync.dma_start(out=rsm, in_=rd16(rs_in[:], 0))
        hn_i = vec.tile([128, 16], i32)
        nc.sync.dma_start(out=hn_i, in_=rd16(eospad[:], 2))
        hp_i = vec.tile([128, 16], i32)
        nc.sync.dma_start(out=hp_i, in_=rd16(eospad[:], 0))
        hn = v16()
        nc.vector.tensor_copy(out=hn, in_=hn_i)
        hp = v16()
        nc.vector.tensor_copy(out=hp, in_=hp_i)

        # LN-folded score fix-up:
        # sn = rs*rsp/512 * (a1 - mup*br - mu*cp1 + mu*mup*s11)
        # sp = rs*rsm/512 * (a2 - mum*br - mu*cm1 + mu*mum*s11)
        def fixup(a, mushift, cshift, rshift):
            q = v16()
            nc.vector.tensor_tensor(q, mushift, br, OP.mult)
            t = v16()
            nc.vector.tensor_tensor(t, a, q, OP.subtract)
            nc.vector.tensor_tensor(q, mu, cshift, OP.mult)
            nc.vector.tensor_tensor(t, t, q, OP.subtract)
            nc.vector.tensor_tensor(q, mu, mushift, OP.mult)
            nc.vector.tensor_scalar(q, q, s11_col, None, OP.mult)
            nc.vector.tensor_tensor(t, t, q, OP.add)
            r = v16()
            nc.vector.tensor_tensor(r, rs, rshift, OP.mult)
            nc.vector.tensor_scalar(r, r, 1.0 / 512.0, None, OP.mult)
            nc.vector.tensor_tensor(t, t, r, OP.mult)
            return t

        sn = fixup(a1, mup, cp1, rsp)
        sp = fixup(a2, mum, cm1, rsm)

        sne = v16()
        nc.vector.select(sne, hn_i, sn, neg9)
        spe = v16()
        nc.vector.select(spe, hp_i, sp, neg9)
        m = v16()
        nc.vector.tensor_tensor(m, sne, spe, OP.max)
        en = v16()
        nc.vector.tensor_tensor(en, sne, m, OP.subtract)
        nc.scalar.activation(en, en, AF.Exp)
        ep = v16()
        nc.vector.tensor_tensor(ep, spe, m, OP.subtract)
        nc.scalar.activation(ep, ep, AF.Exp)
        zs = v16()
        nc.vector.tensor_tensor(zs, en, ep, OP.add)
        rz = v16()
        nc.vector.reciprocal(rz, zs)
        nn = v16()
        nc.vector.tensor_tensor(nn, en, rz, OP.mult)
        npv = v16()
        nc.vector.tensor_tensor(npv, ep, rz, OP.mult)
        # caseB flag u = (1-hn)*(1-hp); blend N with uniform 1/S
        t1 = v16()
        nc.vector.tensor_scalar(t1, hn, -1.0, 1.0, OP.mult, OP.add)
        t2 = v16()
        nc.vector.tensor_scalar(t2, hp, -1.0, 1.0, OP.mult, OP.add)
        cb = v16()
        nc.vector.tensor_tensor(cb, t1, t2, OP.mult)
        omcb = v16()
        nc.vector.tensor_scalar(omcb, cb, -1.0, 1.0, OP.mult, OP.add)
        cbS = v16()
        nc.vector.tensor_scalar(cbS, cb, 1.0 / S, None, OP.mult)
        for nv in (nn, npv):
            nc.vector.tensor_tensor(nv, nv, omcb, OP.mult)
            nc.vector.tensor_tensor(nv, nv, cbS, OP.add)
        # Np shifted by +1 (value at i+1)
        npsh = v16()
        nc.vector.memset(npsh, 0.0)
        nc.vector.tensor_copy(out=npsh[:, 0:15], in_=npv[:, 1:16])
        nc.sync.dma_start(out=npsh[0:127, 15:16], in_=npv[1:128, 0:1])
        msup = v16()
        nc.vector.tensor_tensor(msup, nn, npsh, OP.mult)
        # d_sup = prior + (1-prior)*exp(0.5*ln(msup+1e-9))
        dsup = v16()
        nc.scalar.activation(dsup, msup, AF.Ln, bias=1e-9)
        nc.scalar.activation(dsup, dsup, AF.Exp, scale=0.5)
        nc.vector.tensor_scalar(dsup, dsup, omp_col, pr_col, OP.mult, OP.add)
        # d_main = prior + (1-prior)*(c1 + (c2-c1)*cb)
        dmain = v16()
        nc.vector.tensor_scalar(dmain, cb, C_SBB - C_SQ9, C_SQ9, OP.mult, OP.add)
        nc.vector.tensor_scalar(dmain, dmain, omp_col, pr_col, OP.mult, OP.add)
        # ell, prefix sums
        ell = v16()
        nc.scalar.activation(ell, dsup, AF.Ln, bias=1e-9)
        zv16 = v16()
        nc.vector.memset(zv16, 0.0)
        incl = v16()
        nc.vector.tensor_tensor_scan(incl, ell, zv16, 0.0, OP.add, OP.add)
        excl = v16()
        nc.vector.tensor_tensor(excl, incl, ell, OP.subtract)
        with ExitStack() as p3:
            ps3 = p3.enter_context(tc.tile_pool(name="ps3", bufs=1, space="PSUM"))
            ps_c = ps3.tile([128, 1], f32)
            nc.tensor.matmul(ps_c, lt128, incl[:, 15:16], start=True, stop=True)
            cp_col = col.tile([128, 1], f32)
            nc.vector.tensor_copy(out=cp_col, in_=ps_c)
        cum = v16()
        nc.vector.tensor_scalar(cum, excl, cp_col, None, OP.add)

        def wr16(dtensor, off, src):
            nc.sync.dma_start(
                out=dtensor[off:off + S].rearrange("(p c) -> p c", c=16), in_=src
            )

        wr16(cum_d, 0, cum)
        wr16(dsup_d, 1, dsup)
        wr16(dmain_d, 0, dmain)

        # ============ phase 4: g tiles + band-window strips ============
        with ExitStack() as p4:
            outp = p4.enter_context(tc.tile_pool(name="outp", bufs=3))
            gwin = p4.enter_context(tc.tile_pool(name="gwin", bufs=4))
            strp = p4.enter_context(tc.tile_pool(name="strp", bufs=4))

            cumrow = bigrow.tile([128, S], f32, name="cumrow", tag="cumrow")
            nc.sync.dma_start(out=cumrow, in_=bcast(cum_d[:], S))
            cumcol = col.tile([128, 8], f32)
            nc.sync.dma_start(
                out=cumcol, in_=cum_d[0:HALF].rearrange("(t p) -> p t", p=128)
            )
            negcum = col.tile([128, 8], f32)
            nc.vector.tensor_scalar(negcum, cumcol, -1.0, None, OP.mult)
            dmain_c = col.tile([128, 8], f32)
            nc.sync.dma_start(
                out=dmain_c, in_=dmain_d[0:HALF].rearrange("(t p) -> p t", p=128)
            )
            dsup_c = col.tile([128, 8], f32)
            nc.sync.dma_start(
                out=dsup_c, in_=dsup_d[1:1 + HALF].rearrange("(t p) -> p t", p=128)
            )
            dsupsh_c = col.tile([128, 8], f32)
            nc.sync.dma_start(
                out=dsupsh_c, in_=dsup_d[0:HALF].rearrange("(t p) -> p t", p=128)
            )
            # dsup_d[0] is an uninitialized pad slot; its only consumer is
            # (p=0,t=0) which the sub-diag mask zeroes — but NaN*0=NaN, so
            # scrub it.
            nc.vector.memset(dsupsh_c[0:1, 0:1], 0.0)

            for t in range(NT):
                r0 = t * 128
                gb = outp.tile([128, S], bf16)
                if t > 0:
                    nc.scalar.activation(gb[:, 0:r0], cumrow[:, 0:r0], AF.Exp,
                                         scale=-1.0, bias=cumcol[:, t:t + 1])
                nc.scalar.activation(gb[:, r0 + 128:S], cumrow[:, r0 + 128:S],
                                     AF.Exp, scale=1.0, bias=negcum[:, t:t + 1])
                # window: e2 (lower valid) in place, e1 (upper valid) via mask
                nc.scalar.activation(gb[:, r0:r0 + 128], cumrow[:, r0:r0 + 128],
                                     AF.Exp, scale=-1.0, bias=cumcol[:, t:t + 1])
                e1 = gwin.tile([128, 128], bf16)
                nc.scalar.activation(e1, cumrow[:, r0:r0 + 128], AF.Exp,
                                     scale=1.0, bias=negcum[:, t:t + 1])
                nc.vector.copy_predicated(gb[:, r0:r0 + 128], wup_i, e1)
                dmb = gwin.tile([128, 128], bf16)
                nc.vector.tensor_scalar(dmb, eye_f, dmain_c[:, t:t + 1], None,
                                        OP.mult)
                nc.vector.copy_predicated(gb[:, r0:r0 + 128], eye_i, dmb)
                nc.vector.tensor_scalar(gb, gb, 1.0e-9, None, OP.add)
                nc.sync.dma_start(out=out_g[r0:r0 + 128, :], in_=gb)

                # nb band strip [128,130] at cols [c0, c0+130)
                c0 = r0 - 1 if t > 0 else 0
                mo = 0 if t > 0 else 1
                nbw = strp.tile([128, 130], bf16, name="nbw", tag="nbw")
                nc.vector.tensor_scalar(
                    nbw, urow[:, c0:c0 + 130], ucol_t[:, t:t + 1], v0_col,
                    OP.mult, OP.add
                )
                bv = strp.tile([128, 130], bf16, name="bv", tag="bv")
                nc.vector.tensor_scalar(bv, bmf[:, 0, mo:mo + 130],
                                        dmain_c[:, t:t + 1], None, OP.mult)
                b2 = strp.tile([128, 130], bf16, name="b2", tag="b2")
                nc.vector.tensor_scalar(b2, bmf[:, 1, mo:mo + 130],
                                        dsup_c[:, t:t + 1], None, OP.mult)
                nc.vector.tensor_tensor(bv, bv, b2, OP.add)
                nc.vector.tensor_scalar(b2, bmf[:, 2, mo:mo + 130],
                                        dsupsh_c[:, t:t + 1], None, OP.mult)
                nc.vector.tensor_tensor(bv, bv, b2, OP.add)
                nc.vector.copy_predicated(nbw, bmi[:, mo:mo + 130], bv)
                nc.sync.dma_start(out=out_nb[r0:r0 + 128, c0:c0 + 130], in_=nbw)

    nc.compile()
    return nc


def _consts():
    import ml_dtypes
    k = np.arange(128)
    lt = (k[:, None] < k[None, :]).astype(np.float32)       # lt[k,p]=k<p
    wup_i = (k[None, :] > k[:, None]).astype(np.int32)      # wup[p,w]=w>p
    eye_i = (k[None, :] == k[:, None]).astype(np.int32)
    eye_f = eye_i.astype(np.float32)
    q = np.arange(131)
    eye131 = (q[None, :] == k[:, None] + 1).astype(np.float32)
    sup131 = (q[None, :] == k[:, None] + 2).astype(np.float32)
    sub131 = (q[None, :] == k[:, None]).astype(np.float32)
    bmf = np.stack([eye131, sup131, sub131], axis=1)        # [128,3,131]
    bmi = (eye131 + sup131 + sub131).astype(np.int32)
    ones = np.ones((128, 1), dtype=ml_dtypes.bfloat16)
    return lt, wup_i, eye_i, eye_f, bmf, bmi, ones


def kernel(context, eos_mask, prior, wq, bq, wk, bk, gamma, beta):
    import ml_dtypes
    from concourse.bass_utils import run_bass_kernel_spmd

    if "nc" not in _cache:
        _cache["nc"] = _build()
    nc = _cache["nc"]

    bf = ml_dtypes.bfloat16
    context = np.asarray(context, np.float32)
    eos_mask = np.asarray(eos_mask, np.int32)
    prior = np.asarray(prior, np.float32)
    wqf = np.asarray(wq, np.float32)
    wkf = np.asarray(wk, np.float32)
    lt, wup_i, eye_i, eye_f, bmf, bmi, ones = _consts()

    pr = np.float32(prior[0])
    v0 = pr + (1 - pr) * np.float32(np.sqrt(np.float32(1e-9)))
    vbb = pr + (1 - pr) * np.float32(np.sqrt(np.float32((1.0 / S) ** 2 + 1e-9)))
    dv = np.float32(vbb - v0)

    # LN-fold epilogue constants (host-exact): A = A~^T, A~ = wk^T wq
    # w1 = A 1 = wq^T (wk 1);  w2 = 1^T A = wk^T (wq 1);  s11 = sum(w2)
    w1 = (wqf.T @ wkf.sum(axis=1)).astype(np.float32)
    w2 = (wkf.T @ wqf.sum(axis=1)).astype(np.float32)
    s11 = np.array([w2.sum()], np.float32)
    # per-row LayerNorm stats (exact, f32)
    mu_all = context.mean(axis=2)                      # [B, S]
    var_all = context.var(axis=2)
    rstd_all = 1.0 / np.sqrt(var_all + 1e-5)

    in_maps = []
    for c in range(8):
        b, h = c // 2, c % 2
        x = context[b] if h == 0 else context[b][::-1]
        eo = eos_mask[b] if h == 0 else eos_mask[b][::-1]
        muv = mu_all[b] if h == 0 else mu_all[b][::-1]
        rsv = rstd_all[b] if h == 0 else rstd_all[b][::-1]
        eop = np.zeros(S + 2, np.int32)
        eop[1:S + 1] = eo
        mupad = np.zeros(S + 2, np.float32)
        mupad[1:S + 1] = muv
        rspad = np.zeros(S + 2, np.float32)
        rspad[1:S + 1] = rsv
        u = ((1 - eop[2:S + 2]) * (1 - eop[0:S])).astype(np.float32)
        in_maps.append({
            "x": np.ascontiguousarray(x).astype(bf),
            "eospad": eop,
            "prior": prior, "s11": s11,
            "wq": wqf.astype(bf), "wk": wkf.astype(bf),
            "w1": w1.astype(bf), "w2": w2.astype(bf),
            "mupad": mupad, "rstdpad": rspad,
            "lt128": lt, "wupi": wup_i, "eyei": eye_i, "eyef": eye_f,
            "bmf": bmf, "bmi": bmi, "onesb": ones,
            "usclv": dv * u,
            "ucol": u[0:HALF],
        })

    bkr = run_bass_kernel_spmd(nc, in_maps, core_ids=list(range(8)))
    _cache["last_bkr"] = bkr

    g_out = np.empty((B, S, S), np.float32)
    nb_out = np.empty((B, S, S), np.float32)
    for c in range(8):
        b, h = c // 2, c % 2
        rg = np.asarray(bkr.results[c]["out_g"]).astype(np.float32)
        rn = np.asarray(bkr.results[c]["out_nb"]).astype(np.float32)
        if h == 0:
            g_out[b, :HALF] = rg
            nb_out[b, :HALF] = rn
        else:
            g_out[b, HALF:] = rg[::-1, ::-1]
            nb_out[b, HALF:] = rn[::-1, ::-1]
    return g_out, nb_out


# revision 12
# speedup vs baseline: 1.1770x; 1.0964x over previous
"""GroupAttention sparse-attention kernel for 8 trn2 NeuronCores (v4).

Math (derived + numerically verified against the reference):
  - The tridiagonal mask means each softmax row has >=1 finite entries at
    j=i+-1, or is fully uniform 1/S ("caseB" rows u_i=1).
  - neibor = v0 + (vBB-v0)*u u^T off-band (rank-1), band overwritten with
    d_sup (super/sub) and d_main (diag) via strided diagonal DMAs.
  - g[i,j] = exp(cum[j]-cum[i]) + 1e-9 for j>i (symmetric), diag d_main,
    where cum = exclusive prefix-sum of ell = log(d_sup+1e-9).
  - scores use A~ = wk^T wq and LayerNorm folded into the epilogue:
      xn_i A xn_j = rstd_i rstd_j (xr_i A xr_j - mu_j (xr_i.w1)
                                   - mu_i (w2.xr_j) + mu_i mu_j s11)
    with w1 = A 1, w2 = 1^T A, s11 = 1^T A 1 (A = A~^T), so the device
    only ever touches RAW x — transposes start at t=0 with no LN chain.
    mu/rstd/w1/w2/s11 are computed exactly on the host.
  - A~ and z run in fp8e4 DoubleRow (weights host-scaled by 32 each,
    A~ scale 1024, folded into the final /512 score scale). Scores are
    O(0.03) so fp8's ~3% relative noise is ~1e-3 absolute — negligible.
SPMD: core 2b -> batch b rows [0,1024); core 2b+1 -> batch b reversed
(problem is reversal-covariant), host un-reverses. bq/bk/beta zeros and
gamma ones per the spec, so they are folded away. Outputs are bf16 on
device (tolerance is 2e-2), cast to f32 on host.
"""

import numpy as np
from contextlib import ExitStack

B, S, D = 4, 2048, 1024
NT = 8          # 128-row blocks per core (half of S/128)
HALF = S // 2
WSC = 16.0      # per-weight fp8 scale; A~ carries WSC^2 = 256 (max|A~s|~80 < 240)

_cache = {}


def _build():
    import concourse.bass as bass
    import concourse.bacc as bacc
    import concourse.mybir as mybir
    from concourse.tile import TileContext

    f32 = mybir.dt.float32
    bf16 = mybir.dt.bfloat16
    fp8 = mybir.dt.float8e4
    i32 = mybir.dt.int32
    AF = mybir.ActivationFunctionType
    OP = mybir.AluOpType
    DR = mybir.MatmulPerfMode.DoubleRow

    nc = bacc.Bacc("TRN2", target_bir_lowering=False)

    # ---------------- I/O ----------------
    x_in = nc.dram_tensor("x", [S, D], bf16, kind="ExternalInput")
    eospad = nc.dram_tensor("eospad", [S + 2], i32, kind="ExternalInput")
    prior_t = nc.dram_tensor("prior", [1], f32, kind="ExternalInput")
    s11_t = nc.dram_tensor("s11", [1], f32, kind="ExternalInput")
    wq_in = nc.dram_tensor("wq", [D, D], fp8, kind="ExternalInput")
    wk_in = nc.dram_tensor("wk", [D, D], fp8, kind="ExternalInput")
    w1_in = nc.dram_tensor("w1", [D], bf16, kind="ExternalInput")
    w2_in = nc.dram_tensor("w2", [D], bf16, kind="ExternalInput")
    mu_in = nc.dram_tensor("mupad", [S + 2], f32, kind="ExternalInput")
    rs_in = nc.dram_tensor("rstdpad", [S + 2], f32, kind="ExternalInput")
    lt_in = nc.dram_tensor("lt128", [128, 128], f32, kind="ExternalInput")
    wup_in = nc.dram_tensor("wupi", [128, 128], i32, kind="ExternalInput")
    ones_in = nc.dram_tensor("onesb", [128, 1], bf16, kind="ExternalInput")
    usclv_in = nc.dram_tensor("usclv", [S], f32, kind="ExternalInput")
    ucol_in = nc.dram_tensor("ucol", [HALF], f32, kind="ExternalInput")
    out_nb = nc.dram_tensor("out_nb", [HALF, S], bf16, kind="ExternalOutput")
    out_g = nc.dram_tensor("out_g", [HALF, S], bf16, kind="ExternalOutput")

    C_SQ9 = float(np.sqrt(np.float32(1e-9)))                    # sqrt(1e-9)
    C_SBB = float(np.sqrt(np.float32((1.0 / S) ** 2 + 1e-9)))   # caseB diag sqrt
    SCL = 1.0 / (512.0 * WSC * WSC)

    def bcast(dram_ap, n):
        return bass.AP(tensor=dram_ap.tensor, offset=dram_ap.offset,
                       ap=[[0, 128], [1, n]])

    with TileContext(nc) as tc, ExitStack() as ctx:
        # ---------------- pools (whole-kernel lifetime) ----------------
        consts = ctx.enter_context(tc.tile_pool(name="consts", bufs=1))
        vec = ctx.enter_context(tc.tile_pool(name="vec", bufs=56))
        col = ctx.enter_context(tc.tile_pool(name="col", bufs=8))
        bigrow = ctx.enter_context(tc.tile_pool(name="bigrow", bufs=1))
        at_pool = ctx.enter_context(tc.tile_pool(name="atp", bufs=1))
        xrt_pool = ctx.enter_context(tc.tile_pool(name="xrtp", bufs=1))
        dram = ctx.enter_context(tc.tile_pool(name="dram", bufs=1, space="DRAM"))

        # ---- hot-path DMAs first on the sync ring: weights + transposes ----
        xrT = xrt_pool.tile([128, 8, S], bf16)   # xrT[p,ft,i] = x[i, ft*128+p]
        xr8 = xrt_pool.tile([128, 8, S], fp8)
        wq8 = at_pool.tile([128, 8, D], fp8, name="wq8", tag="wq8")
        wk8 = at_pool.tile([128, 8, D], fp8, name="wk8", tag="wk8")
        for dt in range(8):
            nc.sync.dma_start(out=wk8[:, dt, :],
                              in_=wk_in[dt * 128:(dt + 1) * 128, :])
            nc.sync.dma_start(out=wq8[:, dt, :],
                              in_=wq_in[dt * 128:(dt + 1) * 128, :])
        for ft in range(8):
            nc.sync.dma_start(
                out=xrT[:, ft, :], in_=x_in[:, ft * 128:(ft + 1) * 128],
                transpose=True,
            )

        # ---------------- consts into SBUF (scalar ring) ----------------
        lt128 = consts.tile([128, 128], f32)
        nc.scalar.dma_start(out=lt128, in_=lt_in[:, :])
        wup_i = consts.tile([128, 128], i32)
        nc.scalar.dma_start(out=wup_i, in_=wup_in[:, :])
        ones_b = consts.tile([128, 1], bf16)
        nc.scalar.dma_start(out=ones_b, in_=ones_in[:, :])
        pr_col = consts.tile([128, 1], f32)
        nc.scalar.dma_start(out=pr_col, in_=bcast(prior_t[:], 1))
        s11_col = consts.tile([128, 1], f32)
        nc.scalar.dma_start(out=s11_col, in_=bcast(s11_t[:], 1))
        w1_sb = consts.tile([128, 8], bf16)
        nc.scalar.dma_start(
            out=w1_sb, in_=w1_in[0:D].rearrange("(t p) -> p t", p=128))
        w2_sb = consts.tile([128, 8], bf16)
        nc.scalar.dma_start(
            out=w2_sb, in_=w2_in[0:D].rearrange("(t p) -> p t", p=128))
        omp_col = consts.tile([128, 1], f32)  # 1 - prior
        nc.vector.tensor_scalar(omp_col, pr_col, -1.0, 1.0, OP.mult, OP.add)
        v0_col = consts.tile([128, 1], f32)
        nc.vector.tensor_scalar(v0_col, omp_col, C_SQ9, None, OP.mult)
        nc.vector.tensor_tensor(v0_col, v0_col, pr_col, OP.add)
        neg9 = consts.tile([128, 16], f32)
        nc.vector.memset(neg9, -1.0e9)
        # register const bias columns used by activation(bias=float)
        for ci, cval in enumerate((0.0, 1e-9)):
            cc = consts.tile([128, 1], f32, name=f"cc{ci}", tag=f"cc{ci}")
            nc.vector.memset(cc, cval)
            nc.const_aps.aps[(f32, cval)] = cc[:, :]
        zrow = consts.tile([1, 2], f32)
        nc.vector.memset(zrow, 0.0)

        urow = bigrow.tile([128, S], f32, name="urow", tag="urow")
        nc.scalar.dma_start(out=urow, in_=bcast(usclv_in[:], S))
        ucol_t = col.tile([128, 8], f32, name="ucolt", tag="ucolt")
        nc.scalar.dma_start(
            out=ucol_t, in_=ucol_in[0:HALF].rearrange("(t p) -> p t", p=128)
        )

        # ---------------- DRAM scratch ----------------
        a1_d = dram.tile([S], f32)              # xr_i A xr_{i+1}  (scaled)
        a2_d = dram.tile([S], f32)              # xr_i A xr_{i-1}  (scaled)
        br_d = dram.tile([S], f32)              # xr_i . w1        (scaled)
        cr_d = dram.tile([S + 2], f32)          # [1+i] = w2 . xr_i (scaled)
        cum_d = dram.tile([S], f32)
        dsup16_d = dram.tile([S + 1], bf16)     # [0]=pad, [1+i]=d_sup[i]
        dmain16_d = dram.tile([S], bf16)
        # zero cr_d's pad slots (read via shifted rd16 loads; disjoint from
        # the crow row write, so these can issue early)
        nc.scalar.dma_start(out=cr_d[0:1], in_=zrow[0:1, 0:1])
        nc.scalar.dma_start(out=cr_d[S + 1:S + 2], in_=zrow[0:1, 1:2])

        # ============ phase 3a: early [128,16] vectors (eos/mu/rstd) ========
        def v16(nm="v16"):
            return vec.tile([128, 16], f32, tag="v16", name=nm)

        def rd16(dtensor, off):  # dram vec [off:off+2048] -> [128,16] row-major
            return dtensor[off:off + S].rearrange("(p c) -> p c", c=16)

        mu = v16("mu")
        nc.scalar.dma_start(out=mu, in_=rd16(mu_in[:], 1))
        mup = v16("mup")
        nc.scalar.dma_start(out=mup, in_=rd16(mu_in[:], 2))
        mum = v16("mum")
        nc.scalar.dma_start(out=mum, in_=rd16(mu_in[:], 0))
        rs = v16("rs")
        nc.scalar.dma_start(out=rs, in_=rd16(rs_in[:], 1))
        rsp = v16("rsp")
        nc.scalar.dma_start(out=rsp, in_=rd16(rs_in[:], 2))
        rsm = v16("rsm")
        nc.scalar.dma_start(out=rsm, in_=rd16(rs_in[:], 0))
        hn_i = vec.tile([128, 16], i32)
        nc.scalar.dma_start(out=hn_i, in_=rd16(eospad[:], 2))
        hp_i = vec.tile([128, 16], i32)
        nc.scalar.dma_start(out=hp_i, in_=rd16(eospad[:], 0))
        hn = v16("hn")
        nc.vector.tensor_copy(out=hn, in_=hn_i)
        hp = v16("hp")
        nc.vector.tensor_copy(out=hp, in_=hp_i)
        # caseB flag u = (1-hn)*(1-hp); blend weights
        t1 = v16("t1")
        nc.vector.tensor_scalar(t1, hn, -1.0, 1.0, OP.mult, OP.add)
        t2 = v16("t2")
        nc.vector.tensor_scalar(t2, hp, -1.0, 1.0, OP.mult, OP.add)
        cb = v16("cb")
        nc.vector.tensor_tensor(cb, t1, t2, OP.mult)
        omcb = v16("omcb")
        nc.vector.tensor_scalar(omcb, cb, -1.0, 1.0, OP.mult, OP.add)
        cbS = v16("cbS")
        nc.vector.tensor_scalar(cbS, cb, 1.0 / S, None, OP.mult)
        # d_main = prior + (1-prior)*(c1 + (c2-c1)*cb)  (eos-only -> early)
        dmain = v16("dmain")
        nc.vector.tensor_scalar(dmain, cb, C_SBB - C_SQ9, C_SQ9, OP.mult, OP.add)
        nc.vector.tensor_scalar(dmain, dmain, omp_col, pr_col, OP.mult, OP.add)
        nc.gpsimd.dma_start(out=rd16(dmain16_d, 0), in_=dmain)   # cast f32->bf16
        # rr factors (mu/rstd-only -> early)
        rrn = v16("rrn")
        nc.vector.tensor_tensor(rrn, rs, rsp, OP.mult)
        nc.vector.tensor_scalar(rrn, rrn, SCL, None, OP.mult)
        rrp = v16("rrp")
        nc.vector.tensor_tensor(rrp, rs, rsm, OP.mult)
        nc.vector.tensor_scalar(rrp, rrp, SCL, None, OP.mult)
        q1 = v16("q1")   # mu*mup*s11 reused below
        nc.vector.tensor_tensor(q1, mu, mup, OP.mult)
        nc.vector.tensor_scalar(q1, q1, s11_col, None, OP.mult)
        q2 = v16("q2")
        nc.vector.tensor_tensor(q2, mu, mum, OP.mult)
        nc.vector.tensor_scalar(q2, q2, s11_col, None, OP.mult)
        # sne/spe prefilled with -1e9; predicated-overwritten late
        sne = v16("sne")
        nc.vector.tensor_copy(out=sne, in_=neg9)
        spe = v16("spe")
        nc.vector.tensor_copy(out=spe, in_=neg9)
        npsh = v16("npsh")
        nc.vector.memset(npsh, 0.0)
        zv16 = v16("zv16")
        nc.vector.memset(zv16, 0.0)

        # ============ phase 1: A~^T (fp8 DoubleRow) ============
        with ExitStack() as p1:
            psA = p1.enter_context(tc.tile_pool(name="psA", bufs=2, space="PSUM"))
            at8 = at_pool.tile([128, 8, D], fp8)  # AT[p,ft,e] = A~s[f,e]
            for ft in range(8):
                ps = psA.tile([128, D], f32)
                for dp in range(4):
                    for c in range(2):
                        nc.tensor.matmul(
                            ps[:, c * 512:(c + 1) * 512],
                            wk8[:, 2 * dp:2 * dp + 2, ft * 128:(ft + 1) * 128],
                            wq8[:, 2 * dp:2 * dp + 2, c * 512:(c + 1) * 512],
                            start=(dp == 0),
                            stop=(dp == 3),
                            perf_mode=DR,
                        )
                nc.scalar.copy(out=at8[:, ft, :], in_=ps[:, :])
            # xr8 = fp8 cast of xrT for the z matmuls (split DVE/ACT)
            for ft in range(8):
                if ft % 2 == 0:
                    nc.vector.tensor_copy(out=xr8[:, ft, :], in_=xrT[:, ft, :])
                else:
                    nc.scalar.copy(out=xr8[:, ft, :], in_=xrT[:, ft, :])

        # nb rank-1 tiles (only need eos/prior) — DMA-out slack during z MMs
        with ExitStack() as pnb:
            nbpool = pnb.enter_context(tc.tile_pool(name="nbpool", bufs=3))
            for t in range(NT):
                nb = nbpool.tile([128, S], bf16)
                nc.vector.tensor_scalar(
                    nb, urow, ucol_t[:, t:t + 1], v0_col, OP.mult, OP.add
                )
                nc.sync.dma_start(out=out_nb[t * 128:(t + 1) * 128, :], in_=nb)

        # ============ phase 2: brow/crow; z (fp8 DR); band products ========
        with ExitStack() as p2:
            zpool = p2.enter_context(tc.tile_pool(name="zpool", bufs=2))
            p1pool = p2.enter_context(tc.tile_pool(name="p1pool", bufs=2))
            p2pool = p2.enter_context(tc.tile_pool(name="p2pool", bufs=8))
            rows = p2.enter_context(tc.tile_pool(name="rows", bufs=2))
            psZ = p2.enter_context(tc.tile_pool(name="psZ", bufs=2, space="PSUM"))
            psN = p2.enter_context(tc.tile_pool(name="psN", bufs=1, space="PSUM"))

            # brow = xr.w1 and crow = w2.xr rows (need only xrT + host w1/w2)
            for nm, wcol, dst, doff in (("br", w1_sb, br_d, 0),
                                        ("cr", w2_sb, cr_d, 1)):
                ps_r = psN.tile([1, S], f32, tag="psrow", name=f"ps_{nm}")
                for eb in range(8):
                    for c in range(4):
                        nc.tensor.matmul(
                            ps_r[0:1, c * 512:(c + 1) * 512],
                            wcol[:, eb:eb + 1],
                            xrT[:, eb, c * 512:(c + 1) * 512],
                            start=(eb == 0),
                            stop=(eb == 7),
                        )
                row_r = rows.tile([1, S], f32, tag="rowr", name=f"row_{nm}")
                nc.scalar.copy(out=row_r, in_=ps_r[0:1, :])
                nc.sync.dma_start(out=dst[doff:doff + S], in_=row_r)

            ps_n = psN.tile([1, S], f32, tag="psrow", name="ps_n")
            p2tiles = []
            for et in range(8):
                zb = zpool.tile([128, S], bf16)
                for half in range(2):
                    ps = psZ.tile([128, 1024], f32)
                    for fp in range(4):
                        for c in range(2):
                            off = half * 1024 + c * 512
                            nc.tensor.matmul(
                                ps[:, c * 512:(c + 1) * 512],
                                at8[:, 2 * fp:2 * fp + 2,
                                    et * 128:(et + 1) * 128],
                                xr8[:, 2 * fp:2 * fp + 2, off:off + 512],
                                start=(fp == 0),
                                stop=(fp == 3),
                                perf_mode=DR,
                            )
                    nc.scalar.copy(out=zb[:, half * 1024:(half + 1) * 1024],
                                   in_=ps)
                pt1 = p1pool.tile([128, S], bf16)
                nc.vector.tensor_tensor(
                    pt1[:, 0:S - 1], xrT[:, et, 0:S - 1], zb[:, 1:S], OP.mult
                )
                pt2 = p2pool.tile([128, S], bf16)
                nc.vector.tensor_tensor(
                    pt2[:, 1:S], xrT[:, et, 1:S], zb[:, 0:S - 1], OP.mult
                )
                p2tiles.append(pt2)
                for c in range(4):
                    nc.tensor.matmul(
                        ps_n[0:1, c * 512:(c + 1) * 512],
                        ones_b,
                        pt1[:, c * 512:(c + 1) * 512],
                        start=(et == 0),
                        stop=(et == 7),
                    )
            row_n = rows.tile([1, S], f32, tag="rowr", name="row_n")
            nc.scalar.copy(out=row_n, in_=ps_n[0:1, :])
            nc.sync.dma_start(out=a1_d[:], in_=row_n)

            ps_p = psN.tile([1, S], f32, tag="psrow", name="ps_p")
            for et in range(8):
                for c in range(4):
                    nc.tensor.matmul(
                        ps_p[0:1, c * 512:(c + 1) * 512],
                        ones_b,
                        p2tiles[et][:, c * 512:(c + 1) * 512],
                        start=(et == 0),
                        stop=(et == 7),
                    )
            row_p = rows.tile([1, S], f32, tag="rowr", name="row_p")
            nc.scalar.copy(out=row_p, in_=ps_p[0:1, :])
            nc.sync.dma_start(out=a2_d[:], in_=row_p)

        # ---- early correction terms (need br/cr rows; land mid-z) ----
        br = v16("br")
        nc.scalar.dma_start(out=br, in_=rd16(br_d, 0))
        cp1 = v16("cp1")
        nc.scalar.dma_start(out=cp1, in_=rd16(cr_d, 2))
        cm1 = v16("cm1")
        nc.scalar.dma_start(out=cm1, in_=rd16(cr_d, 0))
        cn = v16("cn")   # mup*br + mu*cp1 - mu*mup*s11
        nc.vector.tensor_tensor(cn, mup, br, OP.mult)
        tq = v16("tq")
        nc.vector.tensor_tensor(tq, mu, cp1, OP.mult)
        nc.vector.tensor_tensor(cn, cn, tq, OP.add)
        nc.vector.tensor_tensor(cn, cn, q1, OP.subtract)
        cp = v16("cp")   # mum*br + mu*cm1 - mu*mum*s11
        nc.vector.tensor_tensor(cp, mum, br, OP.mult)
        nc.vector.tensor_tensor(tq, mu, cm1, OP.mult)
        nc.vector.tensor_tensor(cp, cp, tq, OP.add)
        nc.vector.tensor_tensor(cp, cp, q2, OP.subtract)

        # ============ phase 3b: late chain ============
        a1 = v16("a1")
        nc.scalar.dma_start(out=a1, in_=rd16(a1_d, 0))
        a2 = v16("a2")
        nc.scalar.dma_start(out=a2, in_=rd16(a2_d, 0))
        sn = v16("sn")
        nc.vector.tensor_tensor(sn, a1, cn, OP.subtract)
        nc.vector.tensor_tensor(sn, sn, rrn, OP.mult)
        sp = v16("sp")
        nc.vector.tensor_tensor(sp, a2, cp, OP.subtract)
        nc.vector.tensor_tensor(sp, sp, rrp, OP.mult)
        nc.vector.copy_predicated(sne, hn_i, sn)
        nc.vector.copy_predicated(spe, hp_i, sp)
        m = v16("m")
        nc.vector.tensor_tensor(m, sne, spe, OP.max)
        en = v16("en")
        nc.vector.tensor_tensor(en, sne, m, OP.subtract)
        nc.scalar.activation(en, en, AF.Exp)
        ep = v16("ep")
        nc.vector.tensor_tensor(ep, spe, m, OP.subtract)
        nc.scalar.activation(ep, ep, AF.Exp)
        zs = v16("zs")
        nc.vector.tensor_tensor(zs, en, ep, OP.add)
        rz = v16("rz")
        nc.vector.reciprocal(rz, zs)
        nn = v16("nn")
        nc.vector.tensor_tensor(nn, en, rz, OP.mult)
        npv = v16("npv")
        nc.vector.tensor_tensor(npv, ep, rz, OP.mult)
        for nv in (nn, npv):
            nc.vector.tensor_tensor(nv, nv, omcb, OP.mult)
            nc.vector.tensor_tensor(nv, nv, cbS, OP.add)
        # Np shifted by +1 (value at i+1)
        nc.vector.tensor_copy(out=npsh[:, 0:15], in_=npv[:, 1:16])
        nc.sync.dma_start(out=npsh[0:127, 15:16], in_=npv[1:128, 0:1])
        msup = v16("msup")
        nc.vector.tensor_tensor(msup, nn, npsh, OP.mult)
        # d_sup = prior + (1-prior)*exp(0.5*ln(msup+1e-9))
        dsup = v16("dsup")
        nc.scalar.activation(dsup, msup, AF.Ln, bias=1e-9)
        nc.scalar.activation(dsup, dsup, AF.Exp, scale=0.5)
        nc.vector.tensor_scalar(dsup, dsup, omp_col, pr_col, OP.mult, OP.add)
        nc.gpsimd.dma_start(out=rd16(dsup16_d, 1), in_=dsup)  # cast f32->bf16
        # ell, prefix sums
        ell = v16("ell")
        nc.scalar.activation(ell, dsup, AF.Ln, bias=1e-9)
        incl = v16("incl")
        nc.vector.tensor_tensor_scan(incl, ell, zv16, 0.0, OP.add, OP.add)
        excl = v16("excl")
        nc.vector.tensor_tensor(excl, incl, ell, OP.subtract)
        with ExitStack() as p3:
            ps3 = p3.enter_context(tc.tile_pool(name="ps3", bufs=1, space="PSUM"))
            ps_c = ps3.tile([128, 1], f32)
            nc.tensor.matmul(ps_c, lt128, incl[:, 15:16], start=True, stop=True)
            cp_col = col.tile([128, 1], f32)
            nc.vector.tensor_copy(out=cp_col, in_=ps_c)
        cum = v16("cum")
        nc.vector.tensor_scalar(cum, excl, cp_col, None, OP.add)
        nc.scalar.dma_start(out=rd16(cum_d, 0), in_=cum)

        # ============ phase 4: g tiles; band diagonals via strided DMA ======
        with ExitStack() as p4:
            outp = p4.enter_context(tc.tile_pool(name="outp", bufs=3))
            gwin = p4.enter_context(tc.tile_pool(name="gwin", bufs=4))

            cumrow = bigrow.tile([128, S], f32, name="cumrow", tag="cumrow")
            nc.scalar.dma_start(out=cumrow, in_=bcast(cum_d[:], S))
            cumcol = col.tile([128, 8], f32)
            nc.scalar.dma_start(
                out=cumcol, in_=cum_d[0:HALF].rearrange("(t p) -> p t", p=128)
            )
            negcum = col.tile([128, 8], f32)
            nc.vector.tensor_scalar(negcum, cumcol, -1.0, None, OP.mult)

            for t in range(NT):
                r0 = t * 128
                gb = outp.tile([128, S], bf16)
                if t > 0:
                    nc.scalar.activation(gb[:, 0:r0], cumrow[:, 0:r0], AF.Exp,
                                         scale=-1.0, bias=cumcol[:, t:t + 1])
                nc.scalar.activation(gb[:, r0 + 128:S], cumrow[:, r0 + 128:S],
                                     AF.Exp, scale=1.0, bias=negcum[:, t:t + 1])
                # window: e2 (lower valid) in place, e1 (upper valid) via mask
                nc.scalar.activation(gb[:, r0:r0 + 128], cumrow[:, r0:r0 + 128],
                                     AF.Exp, scale=-1.0, bias=cumcol[:, t:t + 1])
                e1 = gwin.tile([128, 128], bf16)
                nc.scalar.activation(e1, cumrow[:, r0:r0 + 128], AF.Exp,
                                     scale=1.0, bias=negcum[:, t:t + 1])
                nc.vector.copy_predicated(gb[:, r0:r0 + 128], wup_i, e1)
                nc.vector.tensor_scalar(gb, gb, 1.0e-9, None, OP.add)
                nc.sync.dma_start(out=out_g[r0:r0 + 128, :], in_=gb)

            # band diagonals straight into DRAM (strided DRAM->DRAM, bf16)
            def diag_ap(dtt, offset, count):
                return bass.AP(tensor=dtt[:, :].tensor,
                               offset=dtt[:, :].offset + offset,
                               ap=[[S + 1, count]])

            nc.sync.dma_start(out=diag_ap(out_nb, 1, HALF),
                              in_=dsup16_d[1:1 + HALF])
            nc.sync.dma_start(out=diag_ap(out_nb, S, HALF - 1),
                              in_=dsup16_d[1:HALF])
            nc.sync.dma_start(out=diag_ap(out_nb, 0, HALF),
                              in_=dmain16_d[0:HALF])
            nc.sync.dma_start(out=diag_ap(out_g, 0, HALF),
                              in_=dmain16_d[0:HALF])

    nc.compile()
    return nc


def _consts():
    import ml_dtypes
    k = np.arange(128)
    lt = (k[:, None] < k[None, :]).astype(np.float32)       # lt[k,p]=k<p
    wup_i = (k[None, :] > k[:, None]).astype(np.int32)      # wup[p,w]=w>p
    ones = np.ones((128, 1), dtype=ml_dtypes.bfloat16)
    return lt, wup_i, ones


def kernel(context, eos_mask, prior, wq, bq, wk, bk, gamma, beta):
    import ml_dtypes
    from concourse.bass_utils import run_bass_kernel_spmd

    if "nc" not in _cache:
        _cache["nc"] = _build()
    nc = _cache["nc"]

    bf = ml_dtypes.bfloat16
    f8 = ml_dtypes.float8_e4m3
    context = np.asarray(context, np.float32)
    eos_mask = np.asarray(eos_mask, np.int32)
    prior = np.asarray(prior, np.float32)
    wqf = np.asarray(wq, np.float32) * np.float32(WSC)
    wkf = np.asarray(wk, np.float32) * np.float32(WSC)
    lt, wup_i, ones = _consts()

    pr = np.float32(prior[0])
    v0 = pr + (1 - pr) * np.float32(np.sqrt(np.float32(1e-9)))
    vbb = pr + (1 - pr) * np.float32(np.sqrt(np.float32((1.0 / S) ** 2 + 1e-9)))
    dv = np.float32(vbb - v0)

    # LN-fold epilogue constants (host-exact, in the WSC^2 scale): A = A~^T
    # w1 = A 1 = wq^T (wk 1);  w2 = 1^T A = wk^T (wq 1);  s11 = sum(w2)
    w1 = (wqf.T @ wkf.sum(axis=1)).astype(np.float32)
    w2 = (wkf.T @ wqf.sum(axis=1)).astype(np.float32)
    s11 = np.array([w2.sum()], np.float32)
    # per-row LayerNorm stats (exact, f32)
    mu_all = context.mean(axis=2)                      # [B, S]
    var_all = context.var(axis=2)
    rstd_all = 1.0 / np.sqrt(var_all + 1e-5)

    in_maps = []
    for c in range(8):
        b, h = c // 2, c % 2
        x = context[b] if h == 0 else context[b][::-1]
        eo = eos_mask[b] if h == 0 else eos_mask[b][::-1]
        muv = mu_all[b] if h == 0 else mu_all[b][::-1]
        rsv = rstd_all[b] if h == 0 else rstd_all[b][::-1]
        eop = np.zeros(S + 2, np.int32)
        eop[1:S + 1] = eo
        mupad = np.zeros(S + 2, np.float32)
        mupad[1:S + 1] = muv
        rspad = np.zeros(S + 2, np.float32)
        rspad[1:S + 1] = rsv
        u = ((1 - eop[2:S + 2]) * (1 - eop[0:S])).astype(np.float32)
        in_maps.append({
            "x": np.ascontiguousarray(x).astype(bf),
            "eospad": eop,
            "prior": prior, "s11": s11,
            "wq": wqf.astype(f8), "wk": wkf.astype(f8),
            "w1": w1.astype(bf), "w2": w2.astype(bf),
            "mupad": mupad, "rstdpad": rspad,
            "lt128": lt, "wupi": wup_i, "onesb": ones,
            "usclv": dv * u,
            "ucol": u[0:HALF],
        })

    bkr = run_bass_kernel_spmd(nc, in_maps, core_ids=list(range(8)))
    _cache["last_bkr"] = bkr

    g_out = np.empty((B, S, S), np.float32)
    nb_out = np.empty((B, S, S), np.float32)
    for c in range(8):
        b, h = c // 2, c % 2
        rg = np.asarray(bkr.results[c]["out_g"]).astype(np.float32)
        rn = np.asarray(bkr.results[c]["out_nb"]).astype(np.float32)
        if h == 0:
            g_out[b, :HALF] = rg
            nb_out[b, :HALF] = rn
        else:
            g_out[b, HALF:] = rg[::-1, ::-1]
            nb_out[b, HALF:] = rn[::-1, ::-1]
    return g_out, nb_out
